# revision 1
# baseline (speedup 1.0000x reference)
"""Trainium2 Bass kernel for the 10-class supervised-contrastive loss.

Problem shapes (hardcoded): preds [10, 2048, 128] f32, target [2048] int64,
log_vars [10] f32 -> scalar f32.

Sharding (8 cores, SPMD, identical program per core):
  - core c owns class c fully (16 row-strips of 128 rows of the [B,B] matrix)
  - cores 0-3 additionally own a quarter of class 8, cores 4-7 a quarter of
    class 9.  The extra class's rows/labels are fed ROTATED (np.roll) so every
    core statically computes row-strips 0..3 of its "slot 1" class; row sums /
    masked sums are permutation-invariant so rotation is safe, and the
    diagonal stays on the diagonal.

Host prep (O(B*D) layout/scaling only): row-normalize features, cast bf16,
provide both layouts (G = [d, b] transposed, gh = b-tiled [b%128, t*128+d]),
one-hot labels.  ALL O(B^2) work runs on device.

Device, per class:
  per 128-row strip a (slot 0 computes only the upper trapezoid cols >= a*128,
  exploiting symmetry of exp(cos/T)):
      C = G[:,a].T @ G[:, cols]     (bf16 matmuls, f32 PSUM, 512-col chunks)
      zero diagonal window          (DVE mul with (1-I))
      E = Exp(C/T) -> sc (bf16)     (ACT, fused accum_out row-sum)
      col-sums of computed tiles    (PE matmuls E_tile.T @ ones, accumulated
                                     per target strip in one PSUM bank) give
                                     the row-sums of the skipped lower tiles.
  U = gh.T @ [onehot, ones] ; P_a = G_a . u_pos, R_a = G_a . u_all  (PE)
  outputs per row: [rowsum(E), P, R] -> out [128, 60]

Host epilogue (O(B*C)): Z = rowsum - 1 (diag contributed exp(0)=1), masked
mean log-prob from P/R + analytic counts, uncertainty-weighted final sum.
"""

import ml_dtypes
import numpy as np

import concourse.bacc as bacc
import concourse.bass as bass
import concourse.mybir as mybir
import concourse.tile as tile
from concourse.bass_utils import run_bass_kernel_spmd

NUM_CLASSES = 10
B = 2048
D = 128
T = 0.07
BASE_T = 0.07
N_CORES = 8

f32 = mybir.dt.float32
bf16 = mybir.dt.bfloat16
np_bf16 = ml_dtypes.bfloat16

# (slot, row_strip) units every core executes, in order.
UNITS = [(0, rb) for rb in range(16)] + [(1, rb) for rb in range(4)]

TRACE = False
LAST_RESULT = None


def _chunks(c0, c1):
    """Split [c0, c1) at 512-aligned boundaries (PSUM bank limit)."""
    out = []
    c = c0
    while c < c1:
        nxt = min(c1, (c // 512 + 1) * 512)
        out.append((c, nxt))
        c = nxt
    return out


def _build_nc():
    nc = bacc.Bacc(None, target_bir_lowering=False)

    g_dram = [
        nc.dram_tensor(f"g{s}", [128, B], bf16, kind="ExternalInput")
        for s in range(2)
    ]
    gh_dram = [
        nc.dram_tensor(f"gh{s}", [128, B], bf16, kind="ExternalInput")
        for s in range(2)
    ]
    lw_dram = [
        nc.dram_tensor(f"lw{s}", [128, 32], bf16, kind="ExternalInput")
        for s in range(2)
    ]
    masknd_dram = nc.dram_tensor("masknd", [128, 128], f32, kind="ExternalInput")
    onesf_dram = nc.dram_tensor("onesf", [128, 1], f32, kind="ExternalInput")
    out_dram = nc.dram_tensor("out", [128, 3 * len(UNITS)], f32, kind="ExternalOutput")

    add = mybir.AluOpType.add
    EXP = mybir.ActivationFunctionType.Exp

    with tile.TileContext(nc) as tc:
        with (
            tc.tile_pool(name="const", bufs=1) as constp,
            tc.tile_pool(name="gmat", bufs=1) as gmatp,
            tc.tile_pool(name="scp", bufs=4) as scp,
        ):
            masknd_sb = constp.tile([128, 128], f32, tag="masknd")
            nc.sync.dma_start(masknd_sb[:], masknd_dram[:])
            onesf_sb = constp.tile([128, 1], f32, tag="onesf")
            nc.sync.dma_start(onesf_sb[:], onesf_dram[:])
            out_sb = constp.tile([128, 3 * len(UNITS) + 2], f32, tag="out")

            G = []
            GH = []
            LW = []
            for s in range(2):
                g = gmatp.tile([128, B], bf16, tag=f"G{s}", name=f"G{s}")
                nc.sync.dma_start(g[:], g_dram[s][:])
                G.append(g)
                gh = gmatp.tile([128, B], bf16, tag=f"GH{s}", name=f"GH{s}")
                nc.sync.dma_start(gh[:], gh_dram[s][:])
                GH.append(gh)
                lw = gmatp.tile([128, 32], bf16, tag=f"LW{s}", name=f"LW{s}")
                nc.sync.dma_start(lw[:], lw_dram[s][:])
                LW.append(lw)
            Usb = [
                gmatp.tile([128, 2], bf16, tag=f"U{s}", name=f"U{s}") for s in range(2)
            ]

            with (
                tc.tile_pool(name="upsum", bufs=2, space="PSUM") as upp,
                tc.tile_pool(name="prpsum", bufs=2, space="PSUM") as prp,
            ):
                for s in range(2):
                    # U = sum_b gh[b,:]^T * [onehot_b, 1]  (accumulate)
                    up = upp.tile([128, 2], f32, tag="up")
                    for t in range(16):
                        nc.tensor.matmul(
                            up[:],
                            GH[s][:, bass.ts(t, 128)],
                            LW[s][:, bass.ts(t, 2)],
                            start=(t == 0),
                            stop=(t == 15),
                        )
                    nc.vector.tensor_copy(Usb[s][:], up[:])

                # P_a = G_a . u_pos, R_a = G_a . u_all for every row strip.
                for u, (s, rb) in enumerate(UNITS):
                    pr = prp.tile([128, 2], f32, tag="pr")
                    nc.tensor.matmul(
                        pr[:],
                        G[s][:, bass.ts(rb, 128)],
                        Usb[s][:],
                        start=True,
                        stop=True,
                    )
                    nc.vector.tensor_copy(out_sb[:, 3 * u + 1 : 3 * u + 3], pr[:])

            with tc.tile_pool(name="mainpsum", bufs=3, space="PSUM") as cpp:
                # Column-sum accumulator: col rb = sum of E-tile column sums
                # from strips < rb (the transposed/skipped lower tiles).
                csum = cpp.tile([128, 16], f32, tag="csum", bufs=1, name="csum")
                ones_col = onesf_sb[:]  # [128,1] f32 ones

                for u, (s, rb) in enumerate(UNITS):
                    lhsT = G[s][:, bass.ts(rb, 128)]
                    if s == 0:
                        halves = (
                            [(0, rb * 128, 1024), (1024, 1024, 2048)]
                            if rb < 8
                            else [(1024, rb * 128, 2048)]
                        )
                    else:
                        halves = [(0, 0, 1024), (1024, 1024, 2048)]
                    acc_cols = []
                    for hi, (base, c0, c1) in enumerate(halves):
                        cp = cpp.tile([128, 1024], f32, tag="cp", name=f"cp{u}_{hi}")
                        for a0, a1 in _chunks(c0, c1):
                            nc.tensor.matmul(
                                cp[:, a0 - base : a1 - base],
                                lhsT,
                                G[s][:, a0:a1],
                                start=True,
                                stop=True,
                            )
                        if c0 <= rb * 128 < c1:
                            # Zero the diagonal window.
                            w0 = rb * 128 - base
                            nc.vector.tensor_mul(
                                cp[:, w0 : w0 + 128],
                                cp[:, w0 : w0 + 128],
                                masknd_sb[:],
                            )
                        sc = scp.tile([128, 1024], f32, tag="sc", name=f"sc{u}_{hi}")
                        if hi == 0:
                            acol = out_sb[:, 3 * u : 3 * u + 1]
                        else:
                            acol = out_sb[:, 60 + (u % 2) : 61 + (u % 2)]
                        acc_cols.append(acol)
                        nc.scalar.activation(
                            sc[:, c0 - base : c1 - base],
                            cp[:, c0 - base : c1 - base],
                            EXP,
                            scale=1.0 / T,
                            accum_out=acol,
                        )
                        if s == 0:
                            # Column sums of computed tiles feed the row sums
                            # of the mirrored (skipped) tiles.
                            for cb in range(max(rb + 1, c0 // 128), c1 // 128):
                                nc.tensor.matmul(
                                    csum[:, cb : cb + 1],
                                    sc[:, cb * 128 - base : cb * 128 - base + 128],
                                    ones_col,
                                    start=(rb == 0),
                                    stop=(rb == cb - 1),
                                    skip_group_check=True,
                                )
                    if len(acc_cols) == 2:
                        nc.vector.tensor_tensor(
                            out=acc_cols[0], in0=acc_cols[0], in1=acc_cols[1], op=add
                        )
                    if s == 0 and rb > 0:
                        nc.vector.tensor_tensor(
                            out=out_sb[:, 3 * u : 3 * u + 1],
                            in0=out_sb[:, 3 * u : 3 * u + 1],
                            in1=csum[:, rb : rb + 1],
                            op=add,
                        )

            nc.sync.dma_start(out_dram[:], out_sb[:, 0 : 3 * len(UNITS)])
    nc.finalize()
    return nc


_NC_CACHE = None


def _get_nc():
    global _NC_CACHE
    if _NC_CACHE is None:
        _NC_CACHE = _build_nc()
    return _NC_CACHE


def kernel(preds, target, log_vars):
    global LAST_RESULT
    preds = np.asarray(preds, dtype=np.float32)
    target = np.asarray(target)
    log_vars = np.asarray(log_vars, dtype=np.float32)

    onehot = (target[None, :] == np.arange(NUM_CLASSES, dtype=target.dtype)[:, None])
    onehot = onehot.astype(np.float32)  # [10, B]
    npos = onehot.sum(axis=1).astype(np.float64)  # [10]

    # Host prep: row-normalize (f32 stats), cast bf16, build both layouts.
    norms = np.sqrt((preds.astype(np.float32) ** 2).sum(axis=2, dtype=np.float32))
    ghat = (preds / norms[:, :, None]).astype(np_bf16)  # [10, B, D]

    masknd = np.ascontiguousarray(1.0 - np.eye(128, dtype=np.float32))

    in_maps = []
    for c in range(N_CORES):
        cls1 = 8 + c // 4
        off = 512 * (c % 4)
        im = {"masknd": masknd, "onesf": np.ones((128, 1), np.float32)}
        for s, (cls, o) in enumerate([(c, 0), (cls1, off)]):
            gh = np.roll(ghat[cls], -o, axis=0) if o else ghat[cls]
            lab = np.roll(onehot[cls], -o) if o else onehot[cls]
            im[f"g{s}"] = np.ascontiguousarray(gh.T)  # [128, 2048] [d, b]
            im[f"gh{s}"] = np.ascontiguousarray(
                gh.reshape(16, 128, 128).transpose(1, 0, 2).reshape(128, 2048)
            )  # [b%128, t*128+d]
            lw = np.ones((128, 16, 2), dtype=np_bf16)
            lw[:, :, 0] = lab.reshape(16, 128).T
            im[f"lw{s}"] = np.ascontiguousarray(lw.reshape(128, 32))
        in_maps.append(im)

    nc = _get_nc()
    res = run_bass_kernel_spmd(nc, in_maps, list(range(N_CORES)), trace=TRACE)
    LAST_RESULT = res

    # Reassemble per-(class,row) stats.
    zpr = np.zeros((NUM_CLASSES, B, 3), dtype=np.float64)
    rows128 = np.arange(128)
    for c in range(N_CORES):
        o = np.asarray(res.results[c]["out"], dtype=np.float64)  # [128, 60]
        for u, (s, rb) in enumerate(UNITS):
            if s == 0:
                cls, base = c, 0
            else:
                cls, base = 8 + c // 4, 512 * (c % 4)
            rows = (base + rb * 128 + rows128) % B
            zpr[cls, rows, :] = o[:, 3 * u : 3 * u + 3]

    Z = zpr[:, :, 0] - 1.0  # remove diag exp(0)=1 contribution
    P = zpr[:, :, 1]
    R = zpr[:, :, 2]
    lab = onehot.astype(np.float64)
    masked_cos = lab * P + (1.0 - lab) * (R - P)
    masked_logits_sum = (masked_cos - 1.0) / T
    cnt = lab * npos[:, None] + (1.0 - lab) * (B - npos[:, None]) - 1.0
    mlpp = masked_logits_sum / cnt - np.log(Z)
    losses = -(T / BASE_T) * mlpp.mean(axis=1)  # [10]
    lv = log_vars.astype(np.float64)
    final = np.sum(np.exp(-lv) * losses + lv)
    return np.float32(final)



# revision 2
# speedup vs baseline: 2.0148x; 2.0148x over previous
"""Trainium2 Bass kernel for the 10-class supervised-contrastive loss.

Problem shapes (hardcoded): preds [10, 2048, 128] f32, target [2048] int64,
log_vars [10] f32 -> scalar f32.

Sharding (8 cores, SPMD, identical program per core):
  - core c owns class c fully (16 row-strips of 128 rows of the [B,B] matrix)
  - cores 0-3 additionally own a quarter of class 8, cores 4-7 a quarter of
    class 9.  The extra class's rows/labels are fed ROTATED (np.roll) so every
    core statically computes row-strips 0..3 of its "slot 1" class; row sums
    are permutation-invariant so rotation is safe (diagonal stays diagonal).

Device, per class (slot 0 exploits exp-matrix symmetry, slot 1 is full rows):
  per 128-row strip a:
      C = G[:,a].T @ G[:, cols]     (bf16 matmuls, f32 PSUM, 512-col chunks)
      sc = Exp(C/T)                 (ACT, bf16 out to SBUF)
      zero diag window of sc        (DVE bf16 mul with (1-I))
      rowsum(sc) -> out col         (DVE tensor_reduce, f32)
      slot 0 only: colsums of computed tiles via PE with a tiny one-hot
      [128,4] stationary and sc as the 512-wide moving operand, accumulated
      across strips into one persistent [4,512] PSUM bank ("mirror": row j
      holds cols 512j..512j+512 of the flat [2048] mirror vector).  By
      symmetry these colsums are the row sums of the skipped lower tiles.

Host prep (O(B*D)): row-normalize features, cast bf16, transpose.
Host epilogue (O(B*D*C)): Z = rowsum + mirror; P/R per-row pos/all cosine
sums from u-vector matmuls in f64; masked mean log-prob with analytic
counts; uncertainty-weighted final sum.
"""

import ml_dtypes
import numpy as np

import concourse.bacc as bacc
import concourse.bass as bass
import concourse.mybir as mybir
import concourse.tile as tile
from concourse.bass_utils import run_bass_kernel_spmd

NUM_CLASSES = 10
B = 2048
D = 128
T = 0.07
BASE_T = 0.07
N_CORES = 8

f32 = mybir.dt.float32
bf16 = mybir.dt.bfloat16
np_bf16 = ml_dtypes.bfloat16

# (slot, row_strip) units every core executes: 16 strips of its own class,
# 4 strips of the shared class, interleaved for scheduling slack.
UNITS = []
for _a in range(16):
    UNITS.append((0, _a))
    if _a % 4 == 3:
        UNITS.append((1, _a // 4))

TRACE = False
LAST_RESULT = None


def _chunks512(c0, c1):
    """Split [c0, c1) at 512-aligned boundaries (PSUM bank limit)."""
    out = []
    c = c0
    while c < c1:
        nxt = min(c1, (c // 512 + 1) * 512)
        out.append((c, nxt))
        c = nxt
    return out


def _build_nc():
    nc = bacc.Bacc(None, target_bir_lowering=False)

    g_dram = [
        nc.dram_tensor(f"g{s}", [128, B], bf16, kind="ExternalInput")
        for s in range(2)
    ]
    masknd_dram = nc.dram_tensor("masknd", [128, 128], bf16, kind="ExternalInput")
    ones16_dram = nc.dram_tensor("ones16", [128, 16], bf16, kind="ExternalInput")
    out_dram = nc.dram_tensor("out", [128, 20], f32, kind="ExternalOutput")
    mirror_dram = nc.dram_tensor("mirror", [4, 512], f32, kind="ExternalOutput")

    EXP = mybir.ActivationFunctionType.Exp
    X = mybir.AxisListType.X
    add = mybir.AluOpType.add

    with tile.TileContext(nc) as tc:
        with (
            tc.tile_pool(name="const", bufs=1) as constp,
            tc.tile_pool(name="gmat", bufs=1) as gmatp,
            tc.tile_pool(name="scp", bufs=4) as scp,
        ):
            masknd_sb = constp.tile([128, 128], bf16, tag="masknd")
            nc.sync.dma_start(masknd_sb[:], masknd_dram[:])
            ones16_sb = constp.tile([128, 16], bf16, tag="ones16")
            nc.sync.dma_start(ones16_sb[:], ones16_dram[:])
            out_sb = constp.tile([128, 20], f32, tag="out")
            mirror_sb = constp.tile([4, 512], f32, tag="mirror_sb")

            G = []
            for s in range(2):
                g = gmatp.tile([128, B], bf16, tag=f"G{s}", name=f"G{s}")
                nc.sync.dma_start(g[:], g_dram[s][:])
                G.append(g)

            with (
                tc.tile_pool(name="cpsum", bufs=3, space="PSUM") as cpp,
                tc.tile_pool(name="mirpsum", bufs=1, space="PSUM") as mirp,
            ):
                # mirror[j, c] accumulates colsums for flat col 512*j + c.
                mirror = mirp.tile([4, 512], f32, tag="mirror", name="mirror")

                for u, (s, a) in enumerate(UNITS):
                    c0 = 128 * a if s == 0 else 0
                    lhsT = G[s][:, 128 * a : 128 * a + 128]
                    sc = scp.tile([128, B], bf16, tag="sc", name=f"sc{u}")

                    halves = [(c0, 1024), (1024, 2048)] if c0 < 1024 else [(c0, 2048)]
                    for h0, h1 in halves:
                        base = h1 - 1024
                        cp = cpp.tile([128, 1024], f32, tag="cp", name=f"cp{u}_{h0}")
                        for s0, s1 in _chunks512(h0, h1):
                            nc.tensor.matmul(
                                cp[:, s0 - base : s1 - base],
                                lhsT,
                                G[s][:, s0:s1],
                                start=True,
                                stop=True,
                            )
                        nc.scalar.activation(
                            sc[:, h0:h1],
                            cp[:, h0 - base : h1 - base],
                            EXP,
                            scale=1.0 / T,
                        )

                    # Zero the diagonal window (exp(diag) would dwarf Z).
                    w0 = 128 * a
                    nc.vector.tensor_mul(
                        sc[:, w0 : w0 + 128], sc[:, w0 : w0 + 128], masknd_sb[:]
                    )
                    # Row sums of the computed (upper/full) part.
                    nc.vector.tensor_reduce(
                        out_sb[:, u : u + 1],
                        sc[:, c0:B],
                        axis=X,
                        op=add,
                    )

                    if s == 0:
                        # Colsums of computed tiles = rowsums of mirrored
                        # skipped tiles.  Strip 0 covers all buckets
                        # (start=True); its diag-tile cols land in
                        # mirror[:, 0:128], which the host ignores.
                        m0 = 0 if a == 0 else c0 + 128
                        for s0, s1 in _chunks512(m0, 2048):
                            j = s0 // 512
                            nc.tensor.matmul(
                                mirror[:, s0 - 512 * j : s1 - 512 * j],
                                ones16_sb[:, 4 * j : 4 * j + 4],
                                sc[:, s0:s1],
                                start=(a == 0),
                                stop=(a == 4 * j + 2),
                                skip_group_check=True,
                            )

                nc.vector.tensor_copy(mirror_sb[:], mirror[:])

            nc.sync.dma_start(out_dram[:], out_sb[:])
            nc.sync.dma_start(mirror_dram[:], mirror_sb[:])
    nc.finalize()
    return nc


_NC_CACHE = None


def _get_nc():
    global _NC_CACHE
    if _NC_CACHE is None:
        _NC_CACHE = _build_nc()
    return _NC_CACHE


def kernel(preds, target, log_vars):
    global LAST_RESULT
    preds = np.asarray(preds, dtype=np.float32)
    target = np.asarray(target)
    log_vars = np.asarray(log_vars, dtype=np.float32)

    onehot = (target[None, :] == np.arange(NUM_CLASSES, dtype=target.dtype)[:, None])
    onehot = onehot.astype(np.float32)  # [10, B]
    npos = onehot.sum(axis=1).astype(np.float64)  # [10]

    # Host prep: row-normalize (f32 stats), cast bf16 for the device.
    norms = np.sqrt((preds**2).sum(axis=2, dtype=np.float32))
    ghat_f = preds / norms[:, :, None]  # [10, B, D] f32
    ghat = ghat_f.astype(np_bf16)

    masknd = (1.0 - np.eye(128, dtype=np.float32)).astype(np_bf16)
    ones16 = np.zeros((128, 16), dtype=np_bf16)
    for j in range(4):
        ones16[:, 4 * j + j] = 1.0

    in_maps = []
    for c in range(N_CORES):
        cls1 = 8 + c // 4
        off = 512 * (c % 4)
        im = {"masknd": masknd, "ones16": ones16}
        for s, (cls, o) in enumerate([(c, 0), (cls1, off)]):
            gh = np.roll(ghat[cls], -o, axis=0) if o else ghat[cls]
            im[f"g{s}"] = np.ascontiguousarray(gh.T)  # [128, 2048] [d, b]
        in_maps.append(im)

    nc = _get_nc()
    res = run_bass_kernel_spmd(nc, in_maps, list(range(N_CORES)), trace=TRACE)
    LAST_RESULT = res

    # Reassemble per-(class,row) Z (sum over off-diag exp logits).
    Z = np.zeros((NUM_CLASSES, B), dtype=np.float64)
    rows128 = np.arange(128)
    for c in range(N_CORES):
        o = np.asarray(res.results[c]["out"], dtype=np.float64)  # [128, 20]
        mir = np.asarray(res.results[c]["mirror"], dtype=np.float64).reshape(-1)
        mir[0:128] = 0.0
        for u, (s, a) in enumerate(UNITS):
            if s == 0:
                rows = 128 * a + rows128
                Z[c, rows] = o[:, u] + mir[rows]
            else:
                cls, base = 8 + c // 4, 512 * (c % 4)
                rows = (base + 128 * a + rows128) % B
                Z[cls, rows] = o[:, u]

    # P/R per-row positive/total cosine sums (O(B*D*C), f64 on host).
    g64 = ghat_f.astype(np.float64)
    lab = onehot.astype(np.float64)
    u_all = g64.sum(axis=1)  # [10, D]
    u_pos = np.einsum("cbd,cb->cd", g64, lab)  # [10, D]
    P = np.einsum("cbd,cd->cb", g64, u_pos)  # [10, B]
    R = np.einsum("cbd,cd->cb", g64, u_all)  # [10, B]

    masked_cos = lab * P + (1.0 - lab) * (R - P)
    masked_logits_sum = (masked_cos - 1.0) / T
    cnt = lab * npos[:, None] + (1.0 - lab) * (B - npos[:, None]) - 1.0
    mlpp = masked_logits_sum / cnt - np.log(Z)
    losses = -(T / BASE_T) * mlpp.mean(axis=1)  # [10]
    lv = log_vars.astype(np.float64)
    final = np.sum(np.exp(-lv) * losses + lv)
    return np.float32(final)
